# revision 21
# baseline (speedup 1.0000x reference)
"""ChebNet (K=2, 3 layers + global mean pool + linear) on 8 Trainium2 NeuronCores.

Strategy (pull-based graph parallel, v2):
  - Nodes dealt (in-degree balanced) across 8 cores x 98 tiles of 128.
  - Node state kept TRANSPOSED in SBUF (hT [64,128] bf16 per tile).
  - Layers 1,2 are real SpMMs: per layer, y = dinv*(h@Wb) is written per
    AG *piece* (4 row-pieces of ~25 tiles); each piece AllGathers as soon
    as its tiles are prepped, overlapping collectives with compute.
    Messages dma_gather'd (one gather per (4-tile block, piece), 256B rows),
    one-hot built on DVE per 128-slot chunk via is_equal(iota, dst_rel)
    scaled by -dinv[dst] (folds the normalization+sign into the matmul),
    accumulated into PSUM on top of h@Wa + bias, then ReLU.
  - Layer 3 is algebraically eliminated: pooling is linear, so
    sum_{n in g} (-A_hat h2 @ W3b) = (Wp^T z2) with z2 = dinv*(h2@W3b) and
    Wp[n,g] = -cntinv[g] * sum_{e: src=n, batch[dst]=g} dinv[dst] computed
    host-side from graph structure only. Remaining terms pool via
    one-hot(batch)*cntinv matmuls. One [64,64] AllReduce + tiny linear.
"""
import sys

for _p in ("/opt/trn_rl_repo",):
    if _p not in sys.path:
        sys.path.insert(0, _p)

import numpy as np
import ml_dtypes
import concourse.bass as bass
import concourse.mybir as mybir
from concourse import bacc, tile
from concourse.bass_utils import run_bass_kernel_spmd

F32 = mybir.dt.float32
BF16 = mybir.dt.bfloat16
I16 = mybir.dt.int16


class Cfg:
    def __init__(self, N, E, F, H, C, G, ncores=8, block=4):
        self.N, self.E, self.F, self.H, self.C, self.G = N, E, F, H, C, G
        self.ncores = ncores
        npc = -(-N // (ncores * 128)) * 128
        self.NPC = npc
        self.NPAD = npc * ncores
        self.TILES = npc // 128          # 98
        self.BLOCK = block
        # AG pieces: 2 pieces with even tile counts; y rows are PAIRED
        # (two nodes per 256B row), so seg = piece*2 + tile-parity.
        nts = [self.TILES // 2 + self.TILES % 2, self.TILES // 2]
        if nts[0] % 2:
            nts = [nts[0] + 1, nts[1] - 1]
        self.PIECE_NT = nts              # [50, 48]
        self.PIECE_T0 = [0, nts[0]]
        self.PIECE_ROWS = [nt * 128 for nt in nts]
        self.PAIRROWS = [ncores * r // 2 for r in self.PIECE_ROWS]
        assert all(s <= 32767 for s in self.PAIRROWS)
        assert all(nt % 2 == 0 for nt in nts)
        self.NSEG = 4


FULL = Cfg(N=100000, E=1600000, F=64, H=64, C=16, G=64)


# ---------------------------------------------------------------- host prep
def host_prep(cfg, x, edge_index, batch):
    N, G = cfg.N, cfg.G
    ncores, TILES, NPC = cfg.ncores, cfg.TILES, cfg.NPC
    NSEG = cfg.NSEG
    src = np.asarray(edge_index[0], dtype=np.int64)
    dst = np.asarray(edge_index[1], dtype=np.int64)
    batch = np.asarray(batch, dtype=np.int64)

    deg = np.bincount(src, minlength=N).astype(np.float64)
    dinv = np.where(deg > 0, 1.0 / np.sqrt(np.maximum(deg, 1.0)), 0.0).astype(np.float32)

    # ---- deal nodes into (core, tile) bins, balancing in-degree ----
    indeg = np.bincount(dst, minlength=N)
    order = np.argsort(-indeg, kind="stable")
    nbins = ncores * TILES
    k = np.arange(N)
    rnd = k // nbins
    pos = k % nbins
    binid = np.where(rnd % 2 == 0, pos, nbins - 1 - pos)
    core_of_bin = binid % ncores
    tile_of_bin = binid // ncores
    g_of_sorted = core_of_bin * NPC + tile_of_bin * 128 + rnd
    dealt = np.empty(N, dtype=np.int64)
    dealt[order] = g_of_sorted

    src_g = dealt[src]
    dst_g = dealt[dst]

    # per-node (dealt) attributes
    dinv_d = np.zeros(cfg.NPAD, dtype=np.float32)
    dinv_d[dealt] = dinv
    batch_d = np.full(cfg.NPAD, -1.0, dtype=np.float32)
    batch_d[dealt] = batch.astype(np.float32)
    x_d = np.zeros((cfg.NPAD, cfg.F), dtype=np.float32)
    x_d[dealt] = np.asarray(x, dtype=np.float32)

    cnt = np.bincount(batch, minlength=G).astype(np.float32)
    cinv = np.where(cnt > 0, 1.0 / np.maximum(cnt, 1.0), 0.0).astype(np.float32)
    cnt01 = (cnt > 0).astype(np.float32)
    cinv_d = np.zeros(cfg.NPAD, dtype=np.float32)
    bidx = batch_d.astype(np.int64)
    cinv_d[bidx >= 0] = cinv[bidx[bidx >= 0]]

    # ---- edge organization: (dst core, dst tile, src piece) ----
    e_core = dst_g // NPC
    e_tile = (dst_g % NPC) // 128
    e_drel = dst_g % 128
    s_tile = (src_g % NPC) // 128
    s_core = src_g // NPC
    s_slot = src_g % 128
    t0s = np.array(cfg.PIECE_T0)
    e_piece = np.searchsorted(t0s, s_tile, side="right") - 1
    rows_p = np.array(cfg.PIECE_ROWS)[e_piece]
    nt_p = np.array(cfg.PIECE_NT)[e_piece]
    dt0 = s_tile - t0s[e_piece]
    e_seg = e_piece * 2 + dt0 % 2
    # pair-row (slot-major): row holds nodes (slot, 2k) and (slot, 2k+1)
    e_idx = s_core * (rows_p // 2) + s_slot * (nt_p // 2) + dt0 // 2

    order_e = np.lexsort((src_g, e_seg, e_tile, e_core))
    e_core, e_tile, e_drel, e_seg, e_idx = (a[order_e] for a in
                                            (e_core, e_tile, e_drel, e_seg, e_idx))
    dst_go = dst_g[order_e]

    gid = ((e_core * TILES + e_tile) * NSEG + e_seg).astype(np.int64)
    counts = np.bincount(gid, minlength=ncores * TILES * NSEG).reshape(ncores, TILES, NSEG)
    chunk_tbl = -(-counts.max(axis=0) // 128)  # [TILES, NSEG]

    blocks = [list(range(b, min(b + cfg.BLOCK, TILES))) for b in range(0, TILES, cfg.BLOCK)]
    regions = []       # (seg, slot_off, n_slots) -- one per (block, seg), one gather each
    ts_off = np.zeros((TILES, NSEG), dtype=np.int64)
    off = 0
    for blk in blocks:
        for s in range(NSEG):
            g_off = off
            for t in blk:
                ts_off[t, s] = off
                off += int(chunk_tbl[t, s]) * 128
            if off > g_off:
                regions.append((s, g_off, off - g_off))
    TOT = off
    assert TOT % 128 == 0

    # place each core's edges into slots
    idx_all = np.zeros((ncores, TOT), dtype=np.int16)
    drel_all = np.full((ncores, TOT), -1.0, dtype=np.float32)
    grp_start = np.zeros(ncores * TILES * NSEG, dtype=np.int64)
    np.cumsum(counts.reshape(-1)[:-1], out=grp_start[1:])
    within = np.arange(len(gid)) - grp_start[gid]
    slot_of_edge = ts_off[e_tile, e_seg] + within
    for c in range(ncores):
        m = e_core == c
        idx_all[c, slot_of_edge[m]] = e_idx[m].astype(np.int16)
        drel_all[c, slot_of_edge[m]] = e_drel[m].astype(np.float32)

    # wrapped layouts
    idx_wrapped = np.ascontiguousarray(
        np.tile(idx_all.reshape(ncores, TOT // 16, 16).transpose(0, 2, 1), (1, 8, 1))
    )  # [ncores, 128, TOT//16]
    drel_w = drel_all.reshape(ncores, TOT // 128, 128).transpose(0, 2, 1)

    # pool-weight matrix (layer-3 elimination), rows = dealt node ids
    Wp = np.zeros((cfg.NPAD, G), np.float32)
    np.add.at(Wp, (src_g, batch[dst]), -dinv[dst])
    Wp *= cinv[None, :]

    # per-core node-attribute wraps: [128, TILES]
    dinv_wt = dinv_d.reshape(ncores, TILES, 128).transpose(0, 2, 1)
    batch_wt = batch_d.reshape(ncores, TILES, 128).transpose(0, 2, 1)
    cinv_wt = cinv_d.reshape(ncores, TILES, 128).transpose(0, 2, 1)

    plan = dict(chunk_tbl=chunk_tbl, blocks=blocks, regions=regions,
                ts_off=ts_off, TOT=TOT)
    percore = dict(
        xT=[np.ascontiguousarray(x_d[c * NPC:(c + 1) * NPC].T) for c in range(ncores)],
        idx=[np.ascontiguousarray(idx_wrapped[c]) for c in range(ncores)],
        drel=[np.ascontiguousarray(drel_w[c]) for c in range(ncores)],
        Wp=[np.ascontiguousarray(
                Wp[c * NPC:(c + 1) * NPC].reshape(TILES, 128, G).transpose(1, 0, 2)
                .reshape(128, TILES * G)).astype(ml_dtypes.bfloat16)
            for c in range(ncores)],
        dinv=[np.ascontiguousarray(dinv_wt[c]) for c in range(ncores)],
        batch=[np.ascontiguousarray(batch_wt[c]) for c in range(ncores)],
        cinv=[np.ascontiguousarray(cinv_wt[c]) for c in range(ncores)],
    )
    return plan, percore, cnt01


# ---------------------------------------------------------------- program
def build_program(cfg, plan):
    TILES, NSEG, NPC = cfg.TILES, cfg.NSEG, cfg.NPC
    F, H, C, G = cfg.F, cfg.H, cfg.C, cfg.G
    chunk_tbl = plan["chunk_tbl"]; blocks = plan["blocks"]
    regions = plan["regions"]; ts_off = plan["ts_off"]; TOT = plan["TOT"]
    P_T0, P_NT = cfg.PIECE_T0, cfg.PIECE_NT
    PIECE_ROWS, PAIRROWS = cfg.PIECE_ROWS, cfg.PAIRROWS
    piece_of_tile = np.searchsorted(np.array(P_T0), np.arange(TILES), side="right") - 1
    piece_end_tile = [P_T0[p] + P_NT[p] - 1 for p in range(2)]

    # max chunks per (block,seg) region -> fixed msg tile shapes
    CMAXB = {s: 1 for s in range(NSEG)}
    for (s, goff, n) in regions:
        CMAXB[s] = max(CMAXB[s], n // 128)

    nc = bacc.Bacc(num_devices=cfg.ncores, target_bir_lowering=False, num_swdge_queues=4)

    # ---- I/O -----------------------------------------------------------
    P = {}
    P["xT"] = nc.declare_dram_parameter("xT", [F, NPC], BF16, isOutput=False)
    P["idx"] = nc.declare_dram_parameter("idx", [128, TOT // 16], I16, isOutput=False)
    P["drel"] = nc.declare_dram_parameter("drel", [128, TOT // 128], BF16, isOutput=False)
    P["Wp"] = nc.declare_dram_parameter("Wp", [128, TILES * G], BF16, isOutput=False)
    P["dinv"] = nc.declare_dram_parameter("dinv", [128, TILES], F32, isOutput=False)
    P["batch"] = nc.declare_dram_parameter("batch", [128, TILES], F32, isOutput=False)
    P["cinv"] = nc.declare_dram_parameter("cinv", [128, TILES], F32, isOutput=False)
    for l in range(3):
        P[f"Wa{l}"] = nc.declare_dram_parameter(f"Wa{l}", [F if l == 0 else H, H], BF16, isOutput=False)
        P[f"Wb{l}"] = nc.declare_dram_parameter(f"Wb{l}", [F if l == 0 else H, H], BF16, isOutput=False)
        P[f"bias{l}"] = nc.declare_dram_parameter(f"bias{l}", [1, H], BF16, isOutput=False)
    P["Wlin"] = nc.declare_dram_parameter("Wlin", [H, C], F32, isOutput=False)
    P["blin"] = nc.declare_dram_parameter("blin", [1, C], F32, isOutput=False)
    P["cnt01"] = nc.declare_dram_parameter("cnt01", [1, G], BF16, isOutput=False)
    P["iota"] = nc.declare_dram_parameter("iota", [128, 128], BF16, isOutput=False)
    CMX = max(n // 128 for (_s, _o, n) in regions)
    P["iotar"] = nc.declare_dram_parameter("iotar", [128, CMX * 128], BF16, isOutput=False)
    P["identb"] = nc.declare_dram_parameter("identb", [128, 128], BF16, isOutput=False)
    out_ext = nc.declare_dram_parameter("out", [G, C], F32, isOutput=True)

    # internal DRAM: per-piece AG in/out (out double-buffered per layer).
    # y_self is slot-major pair rows [slot, pair, 2H] (one DMA per piece).
    y_self = [nc.dram_tensor(f"y_self{p}", [128, P_NT[p] // 2, 2 * H], BF16)
              for p in range(2)]
    y_piece = [[nc.dram_tensor(f"y_piece{li}_{p}", [PAIRROWS[p], 2 * H], BF16,
                               addr_space="Shared") for p in range(2)]
               for li in range(2)]
    pool_in = nc.dram_tensor("pool_in", [H, G], F32)
    pool_ag = nc.dram_tensor("pool_ag", [cfg.ncores * H, G], F32, addr_space="Shared")

    with tile.TileContext(nc) as tc:
        with tc.tile_pool(name="const", bufs=1) as cpool, \
             tc.tile_pool(name="state", bufs=1) as spool, \
             tc.tile_pool(name="work", bufs=3) as wpool, \
             tc.tile_pool(name="msgs", bufs=2) as mpool, \
             tc.tile_pool(name="oh", bufs=6) as ohpool, \
             tc.tile_pool(name="psS", bufs=2, space="PSUM") as psS, \
             tc.tile_pool(name="psT", bufs=2, space="PSUM") as psT, \
             tc.tile_pool(name="psY", bufs=2, space="PSUM") as psY, \
             tc.tile_pool(name="psPZ", bufs=1, space="PSUM") as psPZ, \
             tc.tile_pool(name="psPH", bufs=1, space="PSUM") as psPH:

            # ---- load constants ----
            def cload(name, shape, dt=F32):
                t = cpool.tile(shape, dt, tag=name)
                nc.sync.dma_start(out=t[:], in_=P[name][:, :])
                return t

            iota_t = cload("iota", [128, 128], BF16)
            identb_t = cload("identb", [128, 128], BF16)
            dinv_t = cload("dinv", [128, TILES])
            ndinv_t = cpool.tile([128, TILES], F32, tag="ndinv")
            nc.vector.tensor_scalar(out=ndinv_t[:], in0=dinv_t[:], scalar1=-1.0,
                                    scalar2=None, op0=mybir.AluOpType.mult)
            batch_t = cload("batch", [128, TILES])
            cinv_t = cload("cinv", [128, TILES])
            drel_t = cload("drel", [128, TOT // 128], BF16)
            cnt01_t = cload("cnt01", [1, G], BF16)
            idx_t = cpool.tile([128, TOT // 16], I16, tag="idx")
            nc.sync.dma_start(out=idx_t[:], in_=P["idx"][:, :])
            iotar_t = cload("iotar", [128, CMX * 128], BF16)
            wp_all = cload("Wp", [128, TILES * G], BF16)
            Wa, Wb, bias = [], [], []
            for l in range(3):
                Wa.append(cload(f"Wa{l}", [F if l == 0 else H, H], BF16))
                Wb.append(cload(f"Wb{l}", [F if l == 0 else H, H], BF16))
                bias.append(cload(f"bias{l}", [1, H], BF16))
            wlin_t = cload("Wlin", [H, C])
            blin_t = cload("blin", [1, C])
            onesb_t = cpool.tile([1, 128], BF16, tag="onesb")
            nc.gpsimd.memset(onesb_t[:], 1.0)
            ones_t = cpool.tile([1, 128], F32, tag="ones")
            nc.gpsimd.memset(ones_t[:], 1.0)

            # persistent transposed node state: layer-0 = x^T (one big load),
            # layer-1 = h1^T written per tile
            hT0_all = cpool.tile([F, TILES * 128], BF16, tag="hT0")
            nc.sync.dma_start(out=hT0_all[:], in_=P["xT"][:, :])
            hT1 = [spool.tile([F, 128], BF16, tag=f"hT1_{t}", name=f"hT1_{t}")
                   for t in range(TILES)]

            def hT_slice(a, t):
                return hT0_all[:, t * 128:(t + 1) * 128] if a == 0 else hT1[t][:]

            psum_pz = psPZ.tile([H, G], F32, tag="pz")
            psum_ph = psPH.tile([H, G], F32, tag="ph")

            NTMX = max(P_NT)
            ysb_state = {"tile": None}

            def y_prep(l, t, hT_ap):
                """y = dinv*(h@Wb[l]) for tile t, staged into a per-piece SBUF
                tile (pair-row layout); one DMA + AllGather per piece."""
                p = int(piece_of_tile[t])
                tt = t - P_T0[p]
                if tt == 0:
                    yp = wpool.tile([128, NTMX * H], BF16, tag="ysbp")
                    ysb_state["tile"] = yp
                yp = ysb_state["tile"]
                col = (tt // 2) * 2 * H + (tt % 2) * H
                ps_y = psY.tile([128, H], F32, tag="y", name="ps_y")
                nc.tensor.matmul(ps_y[:], hT_ap, Wb[l][:], start=True, stop=True)
                nc.scalar.activation(yp[:, col:col + H], ps_y[:],
                                     mybir.ActivationFunctionType.Copy,
                                     scale=dinv_t[:, t:t + 1])
                li = l  # y for SpMM layer l reads buffer set l
                if t == piece_end_tile[p]:
                    nt = P_NT[p]
                    nc.sync.dma_start(out=y_self[p][:, :, :],
                                      in_=yp[:, 0:nt * H])
                    nc.gpsimd.collective_compute(
                        "AllGather", mybir.AluOpType.bypass,
                        replica_groups=[list(range(cfg.ncores))],
                        ins=[y_self[p][:, :, :].opt()],
                        outs=[y_piece[li][p][:, :].opt()],
                    )

            # ---------- L0 prep: y1 pieces straight from x^T ----------
            for t in range(TILES):
                y_prep(0, t, hT0_all[:, t * 128:(t + 1) * 128])

            # ---------- SpMM layers (li = 0, 1) ----------
            for li in range(2):
                ri = 0
                qn = 0
                for blk in blocks:
                    blk_msgs = {}
                    for s in range(NSEG):
                        n_g = sum(int(chunk_tbl[t, s]) * 128 for t in blk)
                        if n_g == 0:
                            continue
                        (rs, roff, rn) = regions[ri]
                        assert rs == s and rn == n_g, (rs, s, rn, n_g, ri)
                        ri += 1
                        nck_r = n_g // 128
                        m_t = mpool.tile([128, CMAXB[s], 2 * H], BF16, tag=f"m{s}")
                        # split into <=1024-slot windows (descriptor carveout:
                        # dynamic_dma_scratch_size // 16 = 1024 descs per queue)
                        nwin = -(-nck_r // 8)
                        base = nck_r // nwin
                        rem = nck_r % nwin
                        w0 = 0
                        for wi in range(nwin):
                            wc = base + (1 if wi < rem else 0)
                            wn = wc * 128
                            woff = roff + w0 * 128
                            nc.gpsimd.dma_gather(
                                m_t[:, w0:w0 + wc, :],
                                y_piece[li][s // 2][0:PAIRROWS[s // 2], :],
                                idx_t[:, woff // 16:(woff + wn) // 16],
                                wn, wn, 2 * H, queue_num=qn)
                            qn = (qn + 1) % 4
                            w0 += wc
                        # one-hot for the whole region in one DVE op:
                        # oh[p, c, j] = (iota[j] == drel[p, c0+c])
                        oh_r = ohpool.tile([128, CMX, 128], BF16, tag="oh")
                        c0 = roff // 128
                        nc.vector.tensor_tensor(
                            out=oh_r[:, 0:nck_r, :],
                            in0=iotar_t[:, 0:nck_r * 128].rearrange(
                                "p (c j) -> p c j", j=128),
                            in1=drel_t[:, c0:c0 + nck_r].unsqueeze(2)
                                .broadcast_to([128, nck_r, 128]),
                            op=mybir.AluOpType.is_equal)
                        blk_msgs[s] = (m_t, oh_r, roff)

                    for t in blk:
                        ps_d = psS.tile([128, H], F32, tag="s", name="ps_d")
                        nc.tensor.matmul(ps_d[:], hT_slice(li, t), Wa[li][:],
                                         start=True, stop=False)
                        nc.tensor.matmul(ps_d[:], onesb_t[:], bias[li][:],
                                         start=False, stop=True)
                        d_sb = wpool.tile([128, H], F32, tag="dsb")
                        nc.scalar.activation(d_sb[:], ps_d[:],
                                             mybir.ActivationFunctionType.Copy)
                        ps_s = psS.tile([128, H], F32, tag="s", name="ps_s")
                        nch = int(chunk_tbl[t].sum())
                        ci = 0
                        for s in range(NSEG):
                            nck = int(chunk_tbl[t, s])
                            if nck == 0:
                                continue
                            m_t, oh_r, roff2 = blk_msgs[s]
                            lo = (int(ts_off[t, s]) - roff2) // 128
                            hof = (s % 2) * H
                            for ck in range(nck):
                                nc.tensor.matmul(
                                    ps_s[:], oh_r[:, lo + ck, :],
                                    m_t[:, lo + ck, hof:hof + H],
                                    start=(ci == 0), stop=(ci == nch - 1))
                                ci += 1
                        h_pre = wpool.tile([128, H], F32, tag="hpre")
                        nc.vector.scalar_tensor_tensor(
                            out=h_pre[:], in0=ps_s[:], scalar=ndinv_t[:, t:t + 1],
                            in1=d_sb[:], op0=mybir.AluOpType.mult,
                            op1=mybir.AluOpType.add)
                        h_sb = wpool.tile([128, H], BF16, tag="hsb")
                        nc.scalar.activation(h_sb[:], h_pre[:],
                                             mybir.ActivationFunctionType.Relu)
                        ps_t = psT.tile([H, 128], BF16, tag="tr", name="ps_t")
                        nc.tensor.transpose(ps_t[:], h_sb[:], identb_t[:])
                        if li == 0:
                            nc.vector.tensor_copy(hT1[t][:], ps_t[:])
                            y_prep(li + 1, t, hT1[t][:])
                        else:
                            # L2 tail: z = dinv*(h2@W3b); pool accumulations
                            hTc = wpool.tile([H, 128], BF16, tag="hTc")
                            nc.vector.tensor_copy(hTc[:], ps_t[:])
                            ps_y = psY.tile([128, H], F32, tag="y", name="ps_y")
                            nc.tensor.matmul(ps_y[:], hTc[:], Wb[2][:],
                                             start=True, stop=True)
                            z_sb = wpool.tile([128, H], BF16, tag="zsb")
                            nc.scalar.activation(z_sb[:], ps_y[:],
                                                 mybir.ActivationFunctionType.Copy,
                                                 scale=dinv_t[:, t:t + 1])
                            poh_t = ohpool.tile([128, G], BF16, tag="poh")
                            nc.vector.tensor_scalar(
                                out=poh_t[:], in0=iota_t[:, :G],
                                scalar1=batch_t[:, t:t + 1],
                                scalar2=cinv_t[:, t:t + 1],
                                op0=mybir.AluOpType.is_equal,
                                op1=mybir.AluOpType.mult)
                            nc.tensor.matmul(psum_ph[:], h_sb[:], poh_t[:],
                                             start=(t == 0), stop=(t == TILES - 1),
                                             skip_group_check=True)
                            nc.tensor.matmul(psum_pz[:], z_sb[:],
                                             wp_all[:, t * G:(t + 1) * G],
                                             start=(t == 0), stop=False,
                                             skip_group_check=True)
                assert ri == len(regions), (ri, len(regions))

            # ---------- finish pooled^T = psum_pz + W3a^T @ P2T + b3 x cnt01 ----------
            p2t_sb = spool.tile([H, G], BF16, tag="p2t")
            nc.vector.tensor_copy(p2t_sb[:], psum_ph[:])
            nc.tensor.matmul(psum_pz[:], Wa[2][:], p2t_sb[:],
                             start=False, stop=False, skip_group_check=True)
            nc.tensor.matmul(psum_pz[:], bias[2][:], cnt01_t[:],
                             start=False, stop=True, skip_group_check=True)
            pl_sb = spool.tile([H, G], F32, tag="plsb")
            nc.vector.tensor_copy(pl_sb[:], psum_pz[:])
            nc.sync.dma_start(out=pool_in[:, :], in_=pl_sb[:])
            nc.gpsimd.collective_compute(
                "AllGather", mybir.AluOpType.bypass,
                replica_groups=[list(range(cfg.ncores))],
                ins=[pool_in[:, :].opt()], outs=[pool_ag[:, :].opt()],
            )
            # local sum over the 8 gathered partials: load as [H, 8, G]
            allp = spool.tile([H, cfg.ncores * G], F32, tag="allp")
            nc.sync.dma_start(
                out=allp[:],
                in_=pool_ag[:, :].rearrange("(i h) g -> h i g", h=H))
            half = spool.tile([H, 4 * G], F32, tag="arH")
            nc.vector.tensor_tensor(
                out=half[:], in0=allp[:, 0:4 * G], in1=allp[:, 4 * G:8 * G],
                op=mybir.AluOpType.add)
            quad = spool.tile([H, 2 * G], F32, tag="arQ")
            nc.vector.tensor_tensor(
                out=quad[:], in0=half[:, 0:2 * G], in1=half[:, 2 * G:4 * G],
                op=mybir.AluOpType.add)
            arT = spool.tile([H, G], F32, tag="arT")
            nc.vector.tensor_tensor(
                out=arT[:], in0=quad[:, 0:G], in1=quad[:, G:2 * G],
                op=mybir.AluOpType.add)
            ps_yo = psY.tile([128, H], F32, tag="y", name="ps_y")
            ps_o = ps_yo[0:G, 0:C]
            nc.tensor.matmul(ps_o, arT[:], wlin_t[:], start=True, stop=False)
            nc.tensor.matmul(ps_o, ones_t[:, :G], blin_t[:], start=False, stop=True)
            out_sb = wpool.tile([G, C], F32, tag="outsb")
            nc.vector.tensor_copy(out_sb[:], ps_o)
            nc.sync.dma_start(out=out_ext[:, :], in_=out_sb[:])

    nc.compile()
    return nc


# ---------------------------------------------------------------- driver
def make_in_maps(cfg, percore, cnt01, cmx, W1, b1, W2, b2, W3, b3, Wlin, blin):
    iota = np.tile(np.arange(128, dtype=np.float32)[None, :], (128, 1))
    ident = np.eye(128, dtype=np.float32)  # identb only
    Ws = [np.asarray(W1, np.float32), np.asarray(W2, np.float32), np.asarray(W3, np.float32)]
    bs = [np.asarray(b1, np.float32), np.asarray(b2, np.float32), np.asarray(b3, np.float32)]
    bf = ml_dtypes.bfloat16
    in_maps = []
    for c in range(cfg.ncores):
        m = {
            "xT": percore["xT"][c].astype(bf),
            "idx": percore["idx"][c],
            "drel": percore["drel"][c].astype(bf),
            "Wp": percore["Wp"][c],
            "dinv": percore["dinv"][c],
            "batch": percore["batch"][c],
            "cinv": percore["cinv"][c],
            "cnt01": cnt01[None, :].astype(bf),
            "iota": iota.astype(bf),
            "iotar": np.tile(iota.astype(bf), (1, cmx)),
            "identb": ident.astype(bf),
            "Wlin": np.ascontiguousarray(Wlin, dtype=np.float32),
            "blin": np.ascontiguousarray(blin, dtype=np.float32)[None, :],
        }
        for l in range(3):
            m[f"Wa{l}"] = np.ascontiguousarray(Ws[l][0]).astype(bf)
            m[f"Wb{l}"] = np.ascontiguousarray(Ws[l][1]).astype(bf)
            m[f"bias{l}"] = np.ascontiguousarray(bs[l])[None, :].astype(bf)
        in_maps.append(m)
    return in_maps


def run(cfg, inputs, trace=False):
    plan, percore, cnt01 = host_prep(cfg, inputs["x"], inputs["edge_index"], inputs["batch"])
    nc = build_program(cfg, plan)
    cmx = max(n // 128 for (_s, _o, n) in plan["regions"])
    in_maps = make_in_maps(cfg, percore, cnt01, cmx,
                           inputs["W1"], inputs["b1"], inputs["W2"], inputs["b2"],
                           inputs["W3"], inputs["b3"], inputs["Wlin"], inputs["blin"])
    res = run_bass_kernel_spmd(nc, in_maps, core_ids=list(range(cfg.ncores)), trace=trace)
    return np.asarray(res.results[0]["out"]), res


def kernel(**inputs) -> np.ndarray:
    out, _ = run(FULL, inputs, trace=False)
    return out


# revision 22
# speedup vs baseline: 1.1475x; 1.1475x over previous
"""ChebNet (K=2, 3 layers + global mean pool + linear) on 8 Trainium2 NeuronCores.

Strategy (pull-based graph parallel, v2):
  - Nodes dealt (in-degree balanced) across 8 cores x 98 tiles of 128.
  - Node state kept TRANSPOSED in SBUF (hT [64,128] bf16 per tile).
  - Layers 1,2 are real SpMMs: per layer, y = dinv*(h@Wb) is written per
    AG *piece* (4 row-pieces of ~25 tiles); each piece AllGathers as soon
    as its tiles are prepped, overlapping collectives with compute.
    Messages dma_gather'd (one gather per (4-tile block, piece), 256B rows),
    one-hot built on DVE per 128-slot chunk via is_equal(iota, dst_rel)
    scaled by -dinv[dst] (folds the normalization+sign into the matmul),
    accumulated into PSUM on top of h@Wa + bias, then ReLU.
  - Layer 3 is algebraically eliminated: pooling is linear, so
    sum_{n in g} (-A_hat h2 @ W3b) = (Wp^T z2) with z2 = dinv*(h2@W3b) and
    Wp[n,g] = -cntinv[g] * sum_{e: src=n, batch[dst]=g} dinv[dst] computed
    host-side from graph structure only. Remaining terms pool via
    one-hot(batch)*cntinv matmuls. One [64,64] AllReduce + tiny linear.
"""
import sys

for _p in ("/opt/trn_rl_repo",):
    if _p not in sys.path:
        sys.path.insert(0, _p)

import numpy as np
import ml_dtypes
import concourse.bass as bass
import concourse.mybir as mybir
from concourse import bacc, tile
from concourse.bass_utils import run_bass_kernel_spmd

F32 = mybir.dt.float32
BF16 = mybir.dt.bfloat16
I16 = mybir.dt.int16


class Cfg:
    def __init__(self, N, E, F, H, C, G, ncores=8, block=4):
        self.N, self.E, self.F, self.H, self.C, self.G = N, E, F, H, C, G
        self.ncores = ncores
        npc = -(-N // (ncores * 128)) * 128
        self.NPC = npc
        self.NPAD = npc * ncores
        self.TILES = npc // 128          # 98
        self.BLOCK = block
        # AG pieces: tile ranges per piece (4 pieces)
        base = self.TILES // 4
        extra = self.TILES % 4
        nts = [base + (1 if i < extra else 0) for i in range(4)]
        self.PIECE_NT = nts              # [25, 25, 24, 24]
        self.PIECE_T0 = [sum(nts[:i]) for i in range(4)]
        self.PIECE_ROWS = [nt * 128 for nt in nts]
        self.SEGROWS = [ncores * r for r in self.PIECE_ROWS]
        assert all(s <= 32767 for s in self.SEGROWS)
        self.NSEG = 4


FULL = Cfg(N=100000, E=1600000, F=64, H=64, C=16, G=64)


# ---------------------------------------------------------------- host prep
def host_prep(cfg, x, edge_index, batch):
    N, G = cfg.N, cfg.G
    ncores, TILES, NPC = cfg.ncores, cfg.TILES, cfg.NPC
    NSEG = cfg.NSEG
    src = np.asarray(edge_index[0], dtype=np.int64)
    dst = np.asarray(edge_index[1], dtype=np.int64)
    batch = np.asarray(batch, dtype=np.int64)

    deg = np.bincount(src, minlength=N).astype(np.float64)
    dinv = np.where(deg > 0, 1.0 / np.sqrt(np.maximum(deg, 1.0)), 0.0).astype(np.float32)

    # ---- deal nodes into (core, tile) bins, balancing in-degree ----
    indeg = np.bincount(dst, minlength=N)
    order = np.argsort(-indeg, kind="stable")
    nbins = ncores * TILES
    k = np.arange(N)
    rnd = k // nbins
    pos = k % nbins
    binid = np.where(rnd % 2 == 0, pos, nbins - 1 - pos)
    core_of_bin = binid % ncores
    tile_of_bin = binid // ncores
    g_of_sorted = core_of_bin * NPC + tile_of_bin * 128 + rnd
    dealt = np.empty(N, dtype=np.int64)
    dealt[order] = g_of_sorted

    src_g = dealt[src]
    dst_g = dealt[dst]

    # per-node (dealt) attributes
    dinv_d = np.zeros(cfg.NPAD, dtype=np.float32)
    dinv_d[dealt] = dinv
    batch_d = np.full(cfg.NPAD, -1.0, dtype=np.float32)
    batch_d[dealt] = batch.astype(np.float32)
    x_d = np.zeros((cfg.NPAD, cfg.F), dtype=np.float32)
    x_d[dealt] = np.asarray(x, dtype=np.float32)

    cnt = np.bincount(batch, minlength=G).astype(np.float32)
    cinv = np.where(cnt > 0, 1.0 / np.maximum(cnt, 1.0), 0.0).astype(np.float32)
    cnt01 = (cnt > 0).astype(np.float32)
    cinv_d = np.zeros(cfg.NPAD, dtype=np.float32)
    bidx = batch_d.astype(np.int64)
    cinv_d[bidx >= 0] = cinv[bidx[bidx >= 0]]

    # ---- edge organization: (dst core, dst tile, src piece) ----
    e_core = dst_g // NPC
    e_tile = (dst_g % NPC) // 128
    e_drel = dst_g % 128
    s_tile = (src_g % NPC) // 128
    s_core = src_g // NPC
    s_slot = src_g % 128
    t0s = np.array(cfg.PIECE_T0)
    e_seg = np.searchsorted(t0s, s_tile, side="right") - 1
    rows_p = np.array(cfg.PIECE_ROWS)[e_seg]
    nt_p = np.array(cfg.PIECE_NT)[e_seg]
    # slot-major within a core's piece block: row = c*rows + slot*nt + (tile-t0)
    e_idx = s_core * rows_p + s_slot * nt_p + (s_tile - t0s[e_seg])

    order_e = np.lexsort((src_g, e_seg, e_tile, e_core))
    e_core, e_tile, e_drel, e_seg, e_idx = (a[order_e] for a in
                                            (e_core, e_tile, e_drel, e_seg, e_idx))
    dst_go = dst_g[order_e]

    gid = ((e_core * TILES + e_tile) * NSEG + e_seg).astype(np.int64)
    counts = np.bincount(gid, minlength=ncores * TILES * NSEG).reshape(ncores, TILES, NSEG)
    chunk_tbl = -(-counts.max(axis=0) // 128)  # [TILES, NSEG]

    blocks = [list(range(b, min(b + cfg.BLOCK, TILES))) for b in range(0, TILES, cfg.BLOCK)]
    regions = []       # (seg, slot_off, n_slots) -- one per (block, seg), one gather each
    ts_off = np.zeros((TILES, NSEG), dtype=np.int64)
    off = 0
    for blk in blocks:
        for s in range(NSEG):
            g_off = off
            for t in blk:
                ts_off[t, s] = off
                off += int(chunk_tbl[t, s]) * 128
            if off > g_off:
                regions.append((s, g_off, off - g_off))
    TOT = off
    assert TOT % 128 == 0

    # place each core's edges into slots
    idx_all = np.zeros((ncores, TOT), dtype=np.int16)
    drel_all = np.full((ncores, TOT), -1.0, dtype=np.float32)
    grp_start = np.zeros(ncores * TILES * NSEG, dtype=np.int64)
    np.cumsum(counts.reshape(-1)[:-1], out=grp_start[1:])
    within = np.arange(len(gid)) - grp_start[gid]
    slot_of_edge = ts_off[e_tile, e_seg] + within
    for c in range(ncores):
        m = e_core == c
        idx_all[c, slot_of_edge[m]] = e_idx[m].astype(np.int16)
        drel_all[c, slot_of_edge[m]] = e_drel[m].astype(np.float32)

    # wrapped layouts
    idx_wrapped = np.ascontiguousarray(
        np.tile(idx_all.reshape(ncores, TOT // 16, 16).transpose(0, 2, 1), (1, 8, 1))
    )  # [ncores, 128, TOT//16]
    drel_w = drel_all.reshape(ncores, TOT // 128, 128).transpose(0, 2, 1)

    # pool-weight matrix (layer-3 elimination), rows = dealt node ids
    Wp = np.zeros((cfg.NPAD, G), np.float32)
    np.add.at(Wp, (src_g, batch[dst]), -dinv[dst])
    Wp *= cinv[None, :]

    # per-core node-attribute wraps: [128, TILES]
    dinv_wt = dinv_d.reshape(ncores, TILES, 128).transpose(0, 2, 1)
    batch_wt = batch_d.reshape(ncores, TILES, 128).transpose(0, 2, 1)
    cinv_wt = cinv_d.reshape(ncores, TILES, 128).transpose(0, 2, 1)

    plan = dict(chunk_tbl=chunk_tbl, blocks=blocks, regions=regions,
                ts_off=ts_off, TOT=TOT)
    percore = dict(
        xT=[np.ascontiguousarray(x_d[c * NPC:(c + 1) * NPC].T) for c in range(ncores)],
        idx=[np.ascontiguousarray(idx_wrapped[c]) for c in range(ncores)],
        drel=[np.ascontiguousarray(drel_w[c]) for c in range(ncores)],
        Wp=[np.ascontiguousarray(
                Wp[c * NPC:(c + 1) * NPC].reshape(TILES, 128, G).transpose(1, 0, 2)
                .reshape(128, TILES * G)).astype(ml_dtypes.bfloat16)
            for c in range(ncores)],
        dinv=[np.ascontiguousarray(dinv_wt[c]) for c in range(ncores)],
        batch=[np.ascontiguousarray(batch_wt[c]) for c in range(ncores)],
        cinv=[np.ascontiguousarray(cinv_wt[c]) for c in range(ncores)],
    )
    return plan, percore, cnt01


# ---------------------------------------------------------------- program
def build_program(cfg, plan):
    TILES, NSEG, NPC = cfg.TILES, cfg.NSEG, cfg.NPC
    F, H, C, G = cfg.F, cfg.H, cfg.C, cfg.G
    chunk_tbl = plan["chunk_tbl"]; blocks = plan["blocks"]
    regions = plan["regions"]; ts_off = plan["ts_off"]; TOT = plan["TOT"]
    P_T0, P_NT = cfg.PIECE_T0, cfg.PIECE_NT
    PIECE_ROWS, SEGROWS = cfg.PIECE_ROWS, cfg.SEGROWS
    piece_of_tile = np.searchsorted(np.array(P_T0), np.arange(TILES), side="right") - 1
    piece_end_tile = [P_T0[p] + P_NT[p] - 1 for p in range(4)]

    # max chunks per (block,seg) region -> fixed msg tile shapes
    CMAXB = {s: 1 for s in range(NSEG)}
    for (s, goff, n) in regions:
        CMAXB[s] = max(CMAXB[s], n // 128)

    nc = bacc.Bacc(num_devices=cfg.ncores, target_bir_lowering=False, num_swdge_queues=4)

    # ---- I/O -----------------------------------------------------------
    P = {}
    P["xT"] = nc.declare_dram_parameter("xT", [F, NPC], BF16, isOutput=False)
    P["idx"] = nc.declare_dram_parameter("idx", [128, TOT // 16], I16, isOutput=False)
    P["drel"] = nc.declare_dram_parameter("drel", [128, TOT // 128], BF16, isOutput=False)
    P["Wp"] = nc.declare_dram_parameter("Wp", [128, TILES * G], BF16, isOutput=False)
    P["dinv"] = nc.declare_dram_parameter("dinv", [128, TILES], F32, isOutput=False)
    P["batch"] = nc.declare_dram_parameter("batch", [128, TILES], F32, isOutput=False)
    P["cinv"] = nc.declare_dram_parameter("cinv", [128, TILES], F32, isOutput=False)
    for l in range(3):
        P[f"Wa{l}"] = nc.declare_dram_parameter(f"Wa{l}", [F if l == 0 else H, H], BF16, isOutput=False)
        P[f"Wb{l}"] = nc.declare_dram_parameter(f"Wb{l}", [F if l == 0 else H, H], BF16, isOutput=False)
        P[f"bias{l}"] = nc.declare_dram_parameter(f"bias{l}", [1, H], BF16, isOutput=False)
    P["Wlin"] = nc.declare_dram_parameter("Wlin", [H, C], F32, isOutput=False)
    P["blin"] = nc.declare_dram_parameter("blin", [1, C], F32, isOutput=False)
    P["cnt01"] = nc.declare_dram_parameter("cnt01", [1, G], BF16, isOutput=False)
    P["iota"] = nc.declare_dram_parameter("iota", [128, 128], BF16, isOutput=False)
    CMX = max(n // 128 for (_s, _o, n) in regions)
    P["iotar"] = nc.declare_dram_parameter("iotar", [128, CMX * 128], BF16, isOutput=False)
    P["identb"] = nc.declare_dram_parameter("identb", [128, 128], BF16, isOutput=False)
    out_ext = nc.declare_dram_parameter("out", [G, C], F32, isOutput=True)

    # internal DRAM: per-piece AG in/out (out double-buffered per layer).
    # y_self is slot-major [slot, tile, 2H] (one DMA per piece).
    y_self = [nc.dram_tensor(f"y_self{p}", [128, P_NT[p], 2 * H], BF16)
              for p in range(4)]
    y_piece = [[nc.dram_tensor(f"y_piece{li}_{p}", [SEGROWS[p], 2 * H], BF16,
                               addr_space="Shared") for p in range(4)]
               for li in range(2)]
    pool_in = nc.dram_tensor("pool_in", [H, G], F32)
    pool_ag = nc.dram_tensor("pool_ag", [cfg.ncores * H, G], F32, addr_space="Shared")

    with tile.TileContext(nc) as tc:
        with tc.tile_pool(name="const", bufs=1) as cpool, \
             tc.tile_pool(name="state", bufs=1) as spool, \
             tc.tile_pool(name="work", bufs=3) as wpool, \
             tc.tile_pool(name="msgs", bufs=2) as mpool, \
             tc.tile_pool(name="oh", bufs=6) as ohpool, \
             tc.tile_pool(name="psS", bufs=2, space="PSUM") as psS, \
             tc.tile_pool(name="psT", bufs=2, space="PSUM") as psT, \
             tc.tile_pool(name="psY", bufs=2, space="PSUM") as psY, \
             tc.tile_pool(name="psPZ", bufs=1, space="PSUM") as psPZ, \
             tc.tile_pool(name="psPH", bufs=1, space="PSUM") as psPH:

            # ---- load constants ----
            def cload(name, shape, dt=F32):
                t = cpool.tile(shape, dt, tag=name)
                nc.sync.dma_start(out=t[:], in_=P[name][:, :])
                return t

            iota_t = cload("iota", [128, 128], BF16)
            identb_t = cload("identb", [128, 128], BF16)
            dinv_t = cload("dinv", [128, TILES])
            ndinv_t = cpool.tile([128, TILES], F32, tag="ndinv")
            nc.vector.tensor_scalar(out=ndinv_t[:], in0=dinv_t[:], scalar1=-1.0,
                                    scalar2=None, op0=mybir.AluOpType.mult)
            batch_t = cload("batch", [128, TILES])
            cinv_t = cload("cinv", [128, TILES])
            drel_t = cload("drel", [128, TOT // 128], BF16)
            cnt01_t = cload("cnt01", [1, G], BF16)
            idx_t = cpool.tile([128, TOT // 16], I16, tag="idx")
            nc.sync.dma_start(out=idx_t[:], in_=P["idx"][:, :])
            iotar_t = cload("iotar", [128, CMX * 128], BF16)
            wp_all = cload("Wp", [128, TILES * G], BF16)
            Wa, Wb, bias = [], [], []
            for l in range(3):
                Wa.append(cload(f"Wa{l}", [F if l == 0 else H, H], BF16))
                Wb.append(cload(f"Wb{l}", [F if l == 0 else H, H], BF16))
                bias.append(cload(f"bias{l}", [1, H], BF16))
            wlin_t = cload("Wlin", [H, C])
            blin_t = cload("blin", [1, C])
            onesb_t = cpool.tile([1, 128], BF16, tag="onesb")
            nc.gpsimd.memset(onesb_t[:], 1.0)
            ones_t = cpool.tile([1, 128], F32, tag="ones")
            nc.gpsimd.memset(ones_t[:], 1.0)

            # persistent transposed node state: layer-0 = x^T (one big load),
            # layer-1 = h1^T written per tile
            hT0_all = cpool.tile([F, TILES * 128], BF16, tag="hT0")
            nc.sync.dma_start(out=hT0_all[:], in_=P["xT"][:, :])
            hT1 = [spool.tile([F, 128], BF16, tag=f"hT1_{t}", name=f"hT1_{t}")
                   for t in range(TILES)]

            def hT_slice(a, t):
                return hT0_all[:, t * 128:(t + 1) * 128] if a == 0 else hT1[t][:]

            psum_pz = psPZ.tile([H, G], F32, tag="pz")
            psum_ph = psPH.tile([H, G], F32, tag="ph")

            NTMX = max(P_NT)
            ysb_state = {"tile": None}

            def y_prep(l, t, hT_ap):
                """y = dinv*(h@Wb[l]) for tile t, staged into a per-piece SBUF
                tile (pair-row layout); one DMA + AllGather per piece."""
                p = int(piece_of_tile[t])
                tt = t - P_T0[p]
                if tt == 0:
                    yp = wpool.tile([128, NTMX * 2 * H], BF16, tag="ysbp")
                    nc.vector.memset(yp[:], 0.0)
                    ysb_state["tile"] = yp
                yp = ysb_state["tile"]
                col = tt * 2 * H
                ps_y = psY.tile([128, H], F32, tag="y", name="ps_y")
                nc.tensor.matmul(ps_y[:], hT_ap, Wb[l][:], start=True, stop=True)
                nc.scalar.activation(yp[:, col:col + H], ps_y[:],
                                     mybir.ActivationFunctionType.Copy,
                                     scale=dinv_t[:, t:t + 1])
                li = l  # y for SpMM layer l reads buffer set l
                if t == piece_end_tile[p]:
                    nt = P_NT[p]
                    nc.sync.dma_start(out=y_self[p][:, :, :],
                                      in_=yp[:, 0:nt * 2 * H])
                    nc.gpsimd.collective_compute(
                        "AllGather", mybir.AluOpType.bypass,
                        replica_groups=[list(range(cfg.ncores))],
                        ins=[y_self[p][:, :, :].opt()],
                        outs=[y_piece[li][p][:, :].opt()],
                    )

            # ---------- L0 prep: y1 pieces straight from x^T ----------
            for t in range(TILES):
                y_prep(0, t, hT0_all[:, t * 128:(t + 1) * 128])

            # ---------- SpMM layers (li = 0, 1) ----------
            for li in range(2):
                ri = 0
                qn = 0
                for blk in blocks:
                    blk_msgs = {}
                    for s in range(NSEG):
                        n_g = sum(int(chunk_tbl[t, s]) * 128 for t in blk)
                        if n_g == 0:
                            continue
                        (rs, roff, rn) = regions[ri]
                        assert rs == s and rn == n_g, (rs, s, rn, n_g, ri)
                        ri += 1
                        nck_r = n_g // 128
                        m_t = mpool.tile([128, CMAXB[s], 2 * H], BF16, tag=f"m{s}")
                        # split into <=1024-slot windows (descriptor carveout:
                        # dynamic_dma_scratch_size // 16 = 1024 descs per queue)
                        nwin = -(-nck_r // 8)
                        base = nck_r // nwin
                        rem = nck_r % nwin
                        w0 = 0
                        for wi in range(nwin):
                            wc = base + (1 if wi < rem else 0)
                            wn = wc * 128
                            woff = roff + w0 * 128
                            nc.gpsimd.dma_gather(
                                m_t[:, w0:w0 + wc, :],
                                y_piece[li][s][0:SEGROWS[s], :],
                                idx_t[:, woff // 16:(woff + wn) // 16],
                                wn, wn, 2 * H, queue_num=qn)
                            qn = (qn + 1) % 4
                            w0 += wc
                        # one-hot for the whole region in one DVE op:
                        # oh[p, c, j] = (iota[j] == drel[p, c0+c])
                        oh_r = ohpool.tile([128, CMX, 128], BF16, tag="oh")
                        c0 = roff // 128
                        nc.vector.tensor_tensor(
                            out=oh_r[:, 0:nck_r, :],
                            in0=iotar_t[:, 0:nck_r * 128].rearrange(
                                "p (c j) -> p c j", j=128),
                            in1=drel_t[:, c0:c0 + nck_r].unsqueeze(2)
                                .broadcast_to([128, nck_r, 128]),
                            op=mybir.AluOpType.is_equal)
                        blk_msgs[s] = (m_t, oh_r, roff)

                    for t in blk:
                        ps_d = psS.tile([128, H], F32, tag="s", name="ps_d")
                        nc.tensor.matmul(ps_d[:], hT_slice(li, t), Wa[li][:],
                                         start=True, stop=False)
                        nc.tensor.matmul(ps_d[:], onesb_t[:], bias[li][:],
                                         start=False, stop=True)
                        d_sb = wpool.tile([128, H], F32, tag="dsb")
                        nc.scalar.activation(d_sb[:], ps_d[:],
                                             mybir.ActivationFunctionType.Copy)
                        ps_s = psS.tile([128, H], F32, tag="s", name="ps_s")
                        nch = int(chunk_tbl[t].sum())
                        ci = 0
                        for s in range(NSEG):
                            nck = int(chunk_tbl[t, s])
                            if nck == 0:
                                continue
                            m_t, oh_r, roff2 = blk_msgs[s]
                            lo = (int(ts_off[t, s]) - roff2) // 128
                            for ck in range(nck):
                                nc.tensor.matmul(
                                    ps_s[:], oh_r[:, lo + ck, :],
                                    m_t[:, lo + ck, 0:H],
                                    start=(ci == 0), stop=(ci == nch - 1))
                                ci += 1
                        h_pre = wpool.tile([128, H], F32, tag="hpre")
                        nc.vector.scalar_tensor_tensor(
                            out=h_pre[:], in0=ps_s[:], scalar=ndinv_t[:, t:t + 1],
                            in1=d_sb[:], op0=mybir.AluOpType.mult,
                            op1=mybir.AluOpType.add)
                        h_sb = wpool.tile([128, H], BF16, tag="hsb")
                        nc.scalar.activation(h_sb[:], h_pre[:],
                                             mybir.ActivationFunctionType.Relu)
                        ps_t = psT.tile([H, 128], BF16, tag="tr", name="ps_t")
                        nc.tensor.transpose(ps_t[:], h_sb[:], identb_t[:])
                        if li == 0:
                            nc.vector.tensor_copy(hT1[t][:], ps_t[:])
                            y_prep(li + 1, t, hT1[t][:])
                        else:
                            # L2 tail: z = dinv*(h2@W3b); pool accumulations
                            hTc = wpool.tile([H, 128], BF16, tag="hTc")
                            nc.vector.tensor_copy(hTc[:], ps_t[:])
                            ps_y = psY.tile([128, H], F32, tag="y", name="ps_y")
                            nc.tensor.matmul(ps_y[:], hTc[:], Wb[2][:],
                                             start=True, stop=True)
                            z_sb = wpool.tile([128, H], BF16, tag="zsb")
                            nc.scalar.activation(z_sb[:], ps_y[:],
                                                 mybir.ActivationFunctionType.Copy,
                                                 scale=dinv_t[:, t:t + 1])
                            poh_t = ohpool.tile([128, G], BF16, tag="poh")
                            nc.vector.tensor_scalar(
                                out=poh_t[:], in0=iota_t[:, :G],
                                scalar1=batch_t[:, t:t + 1],
                                scalar2=cinv_t[:, t:t + 1],
                                op0=mybir.AluOpType.is_equal,
                                op1=mybir.AluOpType.mult)
                            nc.tensor.matmul(psum_ph[:], h_sb[:], poh_t[:],
                                             start=(t == 0), stop=(t == TILES - 1),
                                             skip_group_check=True)
                            nc.tensor.matmul(psum_pz[:], z_sb[:],
                                             wp_all[:, t * G:(t + 1) * G],
                                             start=(t == 0), stop=False,
                                             skip_group_check=True)
                assert ri == len(regions), (ri, len(regions))

            # ---------- finish pooled^T = psum_pz + W3a^T @ P2T + b3 x cnt01 ----------
            p2t_sb = spool.tile([H, G], BF16, tag="p2t")
            nc.vector.tensor_copy(p2t_sb[:], psum_ph[:])
            nc.tensor.matmul(psum_pz[:], Wa[2][:], p2t_sb[:],
                             start=False, stop=False, skip_group_check=True)
            nc.tensor.matmul(psum_pz[:], bias[2][:], cnt01_t[:],
                             start=False, stop=True, skip_group_check=True)
            pl_sb = spool.tile([H, G], F32, tag="plsb")
            nc.vector.tensor_copy(pl_sb[:], psum_pz[:])
            nc.sync.dma_start(out=pool_in[:, :], in_=pl_sb[:])
            nc.gpsimd.collective_compute(
                "AllGather", mybir.AluOpType.bypass,
                replica_groups=[list(range(cfg.ncores))],
                ins=[pool_in[:, :].opt()], outs=[pool_ag[:, :].opt()],
            )
            # local sum over the 8 gathered partials: load as [H, 8, G]
            allp = spool.tile([H, cfg.ncores * G], F32, tag="allp")
            nc.sync.dma_start(
                out=allp[:],
                in_=pool_ag[:, :].rearrange("(i h) g -> h i g", h=H))
            half = spool.tile([H, 4 * G], F32, tag="arH")
            nc.vector.tensor_tensor(
                out=half[:], in0=allp[:, 0:4 * G], in1=allp[:, 4 * G:8 * G],
                op=mybir.AluOpType.add)
            quad = spool.tile([H, 2 * G], F32, tag="arQ")
            nc.vector.tensor_tensor(
                out=quad[:], in0=half[:, 0:2 * G], in1=half[:, 2 * G:4 * G],
                op=mybir.AluOpType.add)
            arT = spool.tile([H, G], F32, tag="arT")
            nc.vector.tensor_tensor(
                out=arT[:], in0=quad[:, 0:G], in1=quad[:, G:2 * G],
                op=mybir.AluOpType.add)
            ps_yo = psY.tile([128, H], F32, tag="y", name="ps_y")
            ps_o = ps_yo[0:G, 0:C]
            nc.tensor.matmul(ps_o, arT[:], wlin_t[:], start=True, stop=False)
            nc.tensor.matmul(ps_o, ones_t[:, :G], blin_t[:], start=False, stop=True)
            out_sb = wpool.tile([G, C], F32, tag="outsb")
            nc.vector.tensor_copy(out_sb[:], ps_o)
            nc.sync.dma_start(out=out_ext[:, :], in_=out_sb[:])

    nc.compile()
    return nc


# ---------------------------------------------------------------- driver
def make_in_maps(cfg, percore, cnt01, cmx, W1, b1, W2, b2, W3, b3, Wlin, blin):
    iota = np.tile(np.arange(128, dtype=np.float32)[None, :], (128, 1))
    ident = np.eye(128, dtype=np.float32)  # identb only
    Ws = [np.asarray(W1, np.float32), np.asarray(W2, np.float32), np.asarray(W3, np.float32)]
    bs = [np.asarray(b1, np.float32), np.asarray(b2, np.float32), np.asarray(b3, np.float32)]
    bf = ml_dtypes.bfloat16
    in_maps = []
    for c in range(cfg.ncores):
        m = {
            "xT": percore["xT"][c].astype(bf),
            "idx": percore["idx"][c],
            "drel": percore["drel"][c].astype(bf),
            "Wp": percore["Wp"][c],
            "dinv": percore["dinv"][c],
            "batch": percore["batch"][c],
            "cinv": percore["cinv"][c],
            "cnt01": cnt01[None, :].astype(bf),
            "iota": iota.astype(bf),
            "iotar": np.tile(iota.astype(bf), (1, cmx)),
            "identb": ident.astype(bf),
            "Wlin": np.ascontiguousarray(Wlin, dtype=np.float32),
            "blin": np.ascontiguousarray(blin, dtype=np.float32)[None, :],
        }
        for l in range(3):
            m[f"Wa{l}"] = np.ascontiguousarray(Ws[l][0]).astype(bf)
            m[f"Wb{l}"] = np.ascontiguousarray(Ws[l][1]).astype(bf)
            m[f"bias{l}"] = np.ascontiguousarray(bs[l])[None, :].astype(bf)
        in_maps.append(m)
    return in_maps


def run(cfg, inputs, trace=False):
    plan, percore, cnt01 = host_prep(cfg, inputs["x"], inputs["edge_index"], inputs["batch"])
    nc = build_program(cfg, plan)
    cmx = max(n // 128 for (_s, _o, n) in plan["regions"])
    in_maps = make_in_maps(cfg, percore, cnt01, cmx,
                           inputs["W1"], inputs["b1"], inputs["W2"], inputs["b2"],
                           inputs["W3"], inputs["b3"], inputs["Wlin"], inputs["blin"])
    res = run_bass_kernel_spmd(nc, in_maps, core_ids=list(range(cfg.ncores)), trace=trace)
    return np.asarray(res.results[0]["out"]), res


def kernel(**inputs) -> np.ndarray:
    out, _ = run(FULL, inputs, trace=False)
    return out


# revision 23
# speedup vs baseline: 1.1770x; 1.0257x over previous
"""ChebNet (K=2, 3 layers + global mean pool + linear) on 8 Trainium2 NeuronCores.

Strategy (pull-based graph parallel, v2):
  - Nodes dealt (in-degree balanced) across 8 cores x 98 tiles of 128.
  - Node state kept TRANSPOSED in SBUF (hT [64,128] bf16 per tile).
  - Layers 1,2 are real SpMMs: per layer, y = dinv*(h@Wb) is written per
    AG *piece* (4 row-pieces of ~25 tiles); each piece AllGathers as soon
    as its tiles are prepped, overlapping collectives with compute.
    Messages dma_gather'd (one gather per (4-tile block, piece), 256B rows),
    one-hot built on DVE per 128-slot chunk via is_equal(iota, dst_rel)
    scaled by -dinv[dst] (folds the normalization+sign into the matmul),
    accumulated into PSUM on top of h@Wa + bias, then ReLU.
  - Layer 3 is algebraically eliminated: pooling is linear, so
    sum_{n in g} (-A_hat h2 @ W3b) = (Wp^T z2) with z2 = dinv*(h2@W3b) and
    Wp[n,g] = -cntinv[g] * sum_{e: src=n, batch[dst]=g} dinv[dst] computed
    host-side from graph structure only. Remaining terms pool via
    one-hot(batch)*cntinv matmuls. One [64,64] AllReduce + tiny linear.
"""
import sys

for _p in ("/opt/trn_rl_repo",):
    if _p not in sys.path:
        sys.path.insert(0, _p)

import numpy as np
import ml_dtypes
import concourse.bass as bass
import concourse.mybir as mybir
from concourse import bacc, tile
from concourse.bass_utils import run_bass_kernel_spmd

F32 = mybir.dt.float32
BF16 = mybir.dt.bfloat16
I16 = mybir.dt.int16


class Cfg:
    def __init__(self, N, E, F, H, C, G, ncores=8, block=4):
        self.N, self.E, self.F, self.H, self.C, self.G = N, E, F, H, C, G
        self.ncores = ncores
        npc = -(-N // (ncores * 128)) * 128
        self.NPC = npc
        self.NPAD = npc * ncores
        self.TILES = npc // 128          # 98
        self.BLOCK = block
        # AG pieces: tile ranges per piece (4 pieces)
        base = self.TILES // 4
        extra = self.TILES % 4
        nts = [base + (1 if i < extra else 0) for i in range(4)]
        self.PIECE_NT = nts              # [25, 25, 24, 24]
        self.PIECE_T0 = [sum(nts[:i]) for i in range(4)]
        self.PIECE_ROWS = [nt * 128 for nt in nts]
        self.SEGROWS = [ncores * r for r in self.PIECE_ROWS]
        assert all(s <= 32767 for s in self.SEGROWS)
        self.NSEG = 4


FULL = Cfg(N=100000, E=1600000, F=64, H=64, C=16, G=64)


# ---------------------------------------------------------------- host prep
def host_prep(cfg, x, edge_index, batch):
    N, G = cfg.N, cfg.G
    ncores, TILES, NPC = cfg.ncores, cfg.TILES, cfg.NPC
    NSEG = cfg.NSEG
    src = np.asarray(edge_index[0], dtype=np.int64)
    dst = np.asarray(edge_index[1], dtype=np.int64)
    batch = np.asarray(batch, dtype=np.int64)

    deg = np.bincount(src, minlength=N).astype(np.float64)
    dinv = np.where(deg > 0, 1.0 / np.sqrt(np.maximum(deg, 1.0)), 0.0).astype(np.float32)

    # ---- deal nodes into (core, tile) bins, balancing in-degree ----
    indeg = np.bincount(dst, minlength=N)
    order = np.argsort(-indeg, kind="stable")
    nbins = ncores * TILES
    k = np.arange(N)
    rnd = k // nbins
    pos = k % nbins
    binid = np.where(rnd % 2 == 0, pos, nbins - 1 - pos)
    core_of_bin = binid % ncores
    tile_of_bin = binid // ncores
    g_of_sorted = core_of_bin * NPC + tile_of_bin * 128 + rnd
    dealt = np.empty(N, dtype=np.int64)
    dealt[order] = g_of_sorted

    src_g = dealt[src]
    dst_g = dealt[dst]

    # per-node (dealt) attributes
    dinv_d = np.zeros(cfg.NPAD, dtype=np.float32)
    dinv_d[dealt] = dinv
    batch_d = np.full(cfg.NPAD, -1.0, dtype=np.float32)
    batch_d[dealt] = batch.astype(np.float32)
    x_d = np.zeros((cfg.NPAD, cfg.F), dtype=np.float32)
    x_d[dealt] = np.asarray(x, dtype=np.float32)

    cnt = np.bincount(batch, minlength=G).astype(np.float32)
    cinv = np.where(cnt > 0, 1.0 / np.maximum(cnt, 1.0), 0.0).astype(np.float32)
    cnt01 = (cnt > 0).astype(np.float32)
    cinv_d = np.zeros(cfg.NPAD, dtype=np.float32)
    bidx = batch_d.astype(np.int64)
    cinv_d[bidx >= 0] = cinv[bidx[bidx >= 0]]

    # ---- edge organization: (dst core, dst tile, src piece) ----
    e_core = dst_g // NPC
    e_tile = (dst_g % NPC) // 128
    e_drel = dst_g % 128
    s_tile = (src_g % NPC) // 128
    s_core = src_g // NPC
    s_slot = src_g % 128
    t0s = np.array(cfg.PIECE_T0)
    e_seg = np.searchsorted(t0s, s_tile, side="right") - 1
    rows_p = np.array(cfg.PIECE_ROWS)[e_seg]
    nt_p = np.array(cfg.PIECE_NT)[e_seg]
    # slot-major within a core's piece block: row = c*rows + slot*nt + (tile-t0)
    e_idx = s_core * rows_p + s_slot * nt_p + (s_tile - t0s[e_seg])

    order_e = np.lexsort((src_g, e_seg, e_tile, e_core))
    e_core, e_tile, e_drel, e_seg, e_idx = (a[order_e] for a in
                                            (e_core, e_tile, e_drel, e_seg, e_idx))
    dst_go = dst_g[order_e]

    gid = ((e_core * TILES + e_tile) * NSEG + e_seg).astype(np.int64)
    counts = np.bincount(gid, minlength=ncores * TILES * NSEG).reshape(ncores, TILES, NSEG)
    chunk_tbl = -(-counts.max(axis=0) // 128)  # [TILES, NSEG]

    blocks = [list(range(b, min(b + cfg.BLOCK, TILES))) for b in range(0, TILES, cfg.BLOCK)]
    regions = []       # (seg, slot_off, n_slots) -- one per (block, seg), one gather each
    ts_off = np.zeros((TILES, NSEG), dtype=np.int64)
    off = 0
    for blk in blocks:
        for s in range(NSEG):
            g_off = off
            for t in blk:
                ts_off[t, s] = off
                off += int(chunk_tbl[t, s]) * 128
            if off > g_off:
                regions.append((s, g_off, off - g_off))
    TOT = off
    assert TOT % 128 == 0

    # place each core's edges into slots
    idx_all = np.zeros((ncores, TOT), dtype=np.int16)
    drel_all = np.full((ncores, TOT), -1.0, dtype=np.float32)
    grp_start = np.zeros(ncores * TILES * NSEG, dtype=np.int64)
    np.cumsum(counts.reshape(-1)[:-1], out=grp_start[1:])
    within = np.arange(len(gid)) - grp_start[gid]
    slot_of_edge = ts_off[e_tile, e_seg] + within
    for c in range(ncores):
        m = e_core == c
        idx_all[c, slot_of_edge[m]] = e_idx[m].astype(np.int16)
        drel_all[c, slot_of_edge[m]] = e_drel[m].astype(np.float32)

    # wrapped layouts
    idx_wrapped = np.ascontiguousarray(
        np.tile(idx_all.reshape(ncores, TOT // 16, 16).transpose(0, 2, 1), (1, 8, 1))
    )  # [ncores, 128, TOT//16]
    drel_w = drel_all.reshape(ncores, TOT // 128, 128).transpose(0, 2, 1)

    # pool-weight matrix (layer-3 elimination), rows = dealt node ids
    Wp = np.zeros((cfg.NPAD, G), np.float32)
    np.add.at(Wp, (src_g, batch[dst]), -dinv[dst])
    Wp *= cinv[None, :]

    # per-core node-attribute wraps: [128, TILES]
    dinv_wt = dinv_d.reshape(ncores, TILES, 128).transpose(0, 2, 1)
    batch_wt = batch_d.reshape(ncores, TILES, 128).transpose(0, 2, 1)
    cinv_wt = cinv_d.reshape(ncores, TILES, 128).transpose(0, 2, 1)
    # pooling one-hot (batch==g)*cinv, wrapped [128, TILES*G] per core
    poh_full = (batch_d[:, None] == np.arange(G)[None, :]) * cinv_d[:, None]
    poh_wt = poh_full.reshape(ncores, TILES, 128, G).transpose(0, 2, 1, 3) \
        .reshape(ncores, 128, TILES * G).astype(ml_dtypes.bfloat16)

    plan = dict(chunk_tbl=chunk_tbl, blocks=blocks, regions=regions,
                ts_off=ts_off, TOT=TOT)
    percore = dict(
        xT=[np.ascontiguousarray(x_d[c * NPC:(c + 1) * NPC].T) for c in range(ncores)],
        idx=[np.ascontiguousarray(idx_wrapped[c]) for c in range(ncores)],
        drel=[np.ascontiguousarray(drel_w[c]) for c in range(ncores)],
        Wp=[np.ascontiguousarray(
                Wp[c * NPC:(c + 1) * NPC].reshape(TILES, 128, G).transpose(1, 0, 2)
                .reshape(128, TILES * G)).astype(ml_dtypes.bfloat16)
            for c in range(ncores)],
        dinv=[np.ascontiguousarray(dinv_wt[c]) for c in range(ncores)],
        batch=[np.ascontiguousarray(batch_wt[c]) for c in range(ncores)],
        cinv=[np.ascontiguousarray(cinv_wt[c]) for c in range(ncores)],
        poh=[np.ascontiguousarray(poh_wt[c]) for c in range(ncores)],
    )
    return plan, percore, cnt01


# ---------------------------------------------------------------- program
def build_program(cfg, plan):
    TILES, NSEG, NPC = cfg.TILES, cfg.NSEG, cfg.NPC
    F, H, C, G = cfg.F, cfg.H, cfg.C, cfg.G
    chunk_tbl = plan["chunk_tbl"]; blocks = plan["blocks"]
    regions = plan["regions"]; ts_off = plan["ts_off"]; TOT = plan["TOT"]
    P_T0, P_NT = cfg.PIECE_T0, cfg.PIECE_NT
    PIECE_ROWS, SEGROWS = cfg.PIECE_ROWS, cfg.SEGROWS
    piece_of_tile = np.searchsorted(np.array(P_T0), np.arange(TILES), side="right") - 1
    piece_end_tile = [P_T0[p] + P_NT[p] - 1 for p in range(4)]

    # max chunks per (block,seg) region -> fixed msg tile shapes
    CMAXB = {s: 1 for s in range(NSEG)}
    for (s, goff, n) in regions:
        CMAXB[s] = max(CMAXB[s], n // 128)

    nc = bacc.Bacc(num_devices=cfg.ncores, target_bir_lowering=False, num_swdge_queues=4)

    # ---- I/O -----------------------------------------------------------
    P = {}
    P["xT"] = nc.declare_dram_parameter("xT", [F, NPC], BF16, isOutput=False)
    P["idx"] = nc.declare_dram_parameter("idx", [128, TOT // 16], I16, isOutput=False)
    P["drel"] = nc.declare_dram_parameter("drel", [128, TOT // 128], BF16, isOutput=False)
    P["Wp"] = nc.declare_dram_parameter("Wp", [128, TILES * G], BF16, isOutput=False)
    P["dinv"] = nc.declare_dram_parameter("dinv", [128, TILES], F32, isOutput=False)
    P["batch"] = nc.declare_dram_parameter("batch", [128, TILES], F32, isOutput=False)
    P["cinv"] = nc.declare_dram_parameter("cinv", [128, TILES], F32, isOutput=False)
    P["poh"] = nc.declare_dram_parameter("poh", [128, TILES * G], BF16, isOutput=False)
    for l in range(3):
        P[f"Wa{l}"] = nc.declare_dram_parameter(f"Wa{l}", [F if l == 0 else H, H], BF16, isOutput=False)
        P[f"Wb{l}"] = nc.declare_dram_parameter(f"Wb{l}", [F if l == 0 else H, H], BF16, isOutput=False)
        P[f"bias{l}"] = nc.declare_dram_parameter(f"bias{l}", [1, H], BF16, isOutput=False)
    P["Wlin"] = nc.declare_dram_parameter("Wlin", [H, C], F32, isOutput=False)
    P["blin"] = nc.declare_dram_parameter("blin", [1, C], F32, isOutput=False)
    P["cnt01"] = nc.declare_dram_parameter("cnt01", [1, G], BF16, isOutput=False)
    P["iota"] = nc.declare_dram_parameter("iota", [128, 128], BF16, isOutput=False)
    CMX = max(n // 128 for (_s, _o, n) in regions)
    P["iotar"] = nc.declare_dram_parameter("iotar", [128, CMX * 128], BF16, isOutput=False)
    P["identb"] = nc.declare_dram_parameter("identb", [128, 128], BF16, isOutput=False)
    out_ext = nc.declare_dram_parameter("out", [G, C], F32, isOutput=True)

    # internal DRAM: per-piece AG in/out (out double-buffered per layer).
    # y_self is slot-major [slot, tile, 2H] (one DMA per piece).
    y_self = [nc.dram_tensor(f"y_self{p}", [128, P_NT[p], 2 * H], BF16)
              for p in range(4)]
    y_piece = [[nc.dram_tensor(f"y_piece{li}_{p}", [SEGROWS[p], 2 * H], BF16,
                               addr_space="Shared") for p in range(4)]
               for li in range(2)]
    pool_in = nc.dram_tensor("pool_in", [H, G], F32)
    pool_ag = nc.dram_tensor("pool_ag", [cfg.ncores * H, G], F32, addr_space="Shared")

    with tile.TileContext(nc) as tc:
        with tc.tile_pool(name="const", bufs=1) as cpool, \
             tc.tile_pool(name="state", bufs=1) as spool, \
             tc.tile_pool(name="work", bufs=3) as wpool, \
             tc.tile_pool(name="msgs", bufs=2) as mpool, \
             tc.tile_pool(name="oh", bufs=6) as ohpool, \
             tc.tile_pool(name="psS", bufs=2, space="PSUM") as psS, \
             tc.tile_pool(name="psT", bufs=2, space="PSUM") as psT, \
             tc.tile_pool(name="psY", bufs=2, space="PSUM") as psY, \
             tc.tile_pool(name="psPZ", bufs=1, space="PSUM") as psPZ, \
             tc.tile_pool(name="psPH", bufs=1, space="PSUM") as psPH:

            # ---- load constants ----
            def cload(name, shape, dt=F32):
                t = cpool.tile(shape, dt, tag=name)
                nc.sync.dma_start(out=t[:], in_=P[name][:, :])
                return t

            iota_t = cload("iota", [128, 128], BF16)
            identb_t = cload("identb", [128, 128], BF16)
            dinv_t = cload("dinv", [128, TILES])
            ndinv_t = cpool.tile([128, TILES], F32, tag="ndinv")
            nc.vector.tensor_scalar(out=ndinv_t[:], in0=dinv_t[:], scalar1=-1.0,
                                    scalar2=None, op0=mybir.AluOpType.mult)
            batch_t = cload("batch", [128, TILES])
            cinv_t = cload("cinv", [128, TILES])
            drel_t = cload("drel", [128, TOT // 128], BF16)
            cnt01_t = cload("cnt01", [1, G], BF16)
            idx_t = cpool.tile([128, TOT // 16], I16, tag="idx")
            nc.sync.dma_start(out=idx_t[:], in_=P["idx"][:, :])
            iotar_t = cload("iotar", [128, CMX * 128], BF16)
            wp_all = cload("Wp", [128, TILES * G], BF16)
            Wa, Wb, bias = [], [], []
            for l in range(3):
                Wa.append(cload(f"Wa{l}", [F if l == 0 else H, H], BF16))
                Wb.append(cload(f"Wb{l}", [F if l == 0 else H, H], BF16))
                bias.append(cload(f"bias{l}", [1, H], BF16))
            wlin_t = cload("Wlin", [H, C])
            blin_t = cload("blin", [1, C])
            onesb_t = cpool.tile([1, 128], BF16, tag="onesb")
            nc.gpsimd.memset(onesb_t[:], 1.0)
            ones_t = cpool.tile([1, 128], F32, tag="ones")
            nc.gpsimd.memset(ones_t[:], 1.0)

            # persistent transposed node state: layer-0 = x^T (one big load),
            # layer-1 = h1^T written per tile
            hT0_all = cpool.tile([F, TILES * 128], BF16, tag="hT0")
            nc.sync.dma_start(out=hT0_all[:], in_=P["xT"][:, :])
            hT1 = [spool.tile([F, 128], BF16, tag=f"hT1_{t}", name=f"hT1_{t}")
                   for t in range(TILES)]

            def hT_slice(a, t):
                return hT0_all[:, t * 128:(t + 1) * 128] if a == 0 else hT1[t][:]

            psum_pz = psPZ.tile([H, G], F32, tag="pz")
            psum_ph = psPH.tile([H, G], F32, tag="ph")

            NTMX = max(P_NT)
            ysb_state = {"tile": None}

            def y_prep(l, t, hT_ap):
                """y = dinv*(h@Wb[l]) for tile t, staged into a per-piece SBUF
                tile (pair-row layout); one DMA + AllGather per piece."""
                p = int(piece_of_tile[t])
                tt = t - P_T0[p]
                if tt == 0:
                    yp = wpool.tile([128, NTMX * 2 * H], BF16, tag="ysbp")
                    nc.vector.memset(yp[:], 0.0)
                    ysb_state["tile"] = yp
                yp = ysb_state["tile"]
                col = tt * 2 * H
                ps_y = psY.tile([128, H], F32, tag="y", name="ps_y")
                nc.tensor.matmul(ps_y[:], hT_ap, Wb[l][:], start=True, stop=True)
                nc.scalar.activation(yp[:, col:col + H], ps_y[:],
                                     mybir.ActivationFunctionType.Copy,
                                     scale=dinv_t[:, t:t + 1])
                li = l  # y for SpMM layer l reads buffer set l
                if t == piece_end_tile[p]:
                    nt = P_NT[p]
                    nc.sync.dma_start(out=y_self[p][:, :, :],
                                      in_=yp[:, 0:nt * 2 * H])
                    nc.gpsimd.collective_compute(
                        "AllGather", mybir.AluOpType.bypass,
                        replica_groups=[list(range(cfg.ncores))],
                        ins=[y_self[p][:, :, :].opt()],
                        outs=[y_piece[li][p][:, :].opt()],
                    )

            # ---------- L0 prep: y1 pieces straight from x^T ----------
            for t in range(TILES):
                y_prep(0, t, hT0_all[:, t * 128:(t + 1) * 128])

            # ---------- SpMM layers (li = 0, 1) ----------
            for li in range(2):
                ri = 0
                qn = 0
                for blk in blocks:
                    blk_msgs = {}
                    for s in range(NSEG):
                        n_g = sum(int(chunk_tbl[t, s]) * 128 for t in blk)
                        if n_g == 0:
                            continue
                        (rs, roff, rn) = regions[ri]
                        assert rs == s and rn == n_g, (rs, s, rn, n_g, ri)
                        ri += 1
                        nck_r = n_g // 128
                        m_t = mpool.tile([128, CMAXB[s], 2 * H], BF16, tag=f"m{s}")
                        # split into <=1024-slot windows (descriptor carveout:
                        # dynamic_dma_scratch_size // 16 = 1024 descs per queue)
                        nwin = -(-nck_r // 8)
                        base = nck_r // nwin
                        rem = nck_r % nwin
                        w0 = 0
                        for wi in range(nwin):
                            wc = base + (1 if wi < rem else 0)
                            wn = wc * 128
                            woff = roff + w0 * 128
                            nc.gpsimd.dma_gather(
                                m_t[:, w0:w0 + wc, :],
                                y_piece[li][s][0:SEGROWS[s], :],
                                idx_t[:, woff // 16:(woff + wn) // 16],
                                wn, wn, 2 * H, queue_num=qn)
                            qn = (qn + 1) % 4
                            w0 += wc
                        # one-hot for the whole region in one DVE op:
                        # oh[p, c, j] = (iota[j] == drel[p, c0+c])
                        oh_r = ohpool.tile([128, CMX, 128], BF16, tag="oh")
                        c0 = roff // 128
                        nc.vector.tensor_tensor(
                            out=oh_r[:, 0:nck_r, :],
                            in0=iotar_t[:, 0:nck_r * 128].rearrange(
                                "p (c j) -> p c j", j=128),
                            in1=drel_t[:, c0:c0 + nck_r].unsqueeze(2)
                                .broadcast_to([128, nck_r, 128]),
                            op=mybir.AluOpType.is_equal)
                        blk_msgs[s] = (m_t, oh_r, roff)

                    for t in blk:
                        ps_d = psS.tile([128, H], F32, tag="s", name="ps_d")
                        nc.tensor.matmul(ps_d[:], hT_slice(li, t), Wa[li][:],
                                         start=True, stop=False)
                        nc.tensor.matmul(ps_d[:], onesb_t[:], bias[li][:],
                                         start=False, stop=True)
                        d_sb = wpool.tile([128, H], F32, tag="dsb")
                        nc.scalar.activation(d_sb[:], ps_d[:],
                                             mybir.ActivationFunctionType.Copy)
                        ps_s = psS.tile([128, H], F32, tag="s", name="ps_s")
                        nch = int(chunk_tbl[t].sum())
                        ci = 0
                        for s in range(NSEG):
                            nck = int(chunk_tbl[t, s])
                            if nck == 0:
                                continue
                            m_t, oh_r, roff2 = blk_msgs[s]
                            lo = (int(ts_off[t, s]) - roff2) // 128
                            for ck in range(nck):
                                nc.tensor.matmul(
                                    ps_s[:], oh_r[:, lo + ck, :],
                                    m_t[:, lo + ck, 0:H],
                                    start=(ci == 0), stop=(ci == nch - 1))
                                ci += 1
                        h_pre = wpool.tile([128, H], F32, tag="hpre")
                        nc.vector.scalar_tensor_tensor(
                            out=h_pre[:], in0=ps_s[:], scalar=ndinv_t[:, t:t + 1],
                            in1=d_sb[:], op0=mybir.AluOpType.mult,
                            op1=mybir.AluOpType.add)
                        h_sb = wpool.tile([128, H], BF16, tag="hsb")
                        nc.scalar.activation(h_sb[:], h_pre[:],
                                             mybir.ActivationFunctionType.Relu)
                        ps_t = psT.tile([H, 128], BF16, tag="tr", name="ps_t")
                        nc.tensor.transpose(ps_t[:], h_sb[:], identb_t[:])
                        if li == 0:
                            nc.scalar.activation(hT1[t][:], ps_t[:],
                                                 mybir.ActivationFunctionType.Copy)
                            y_prep(li + 1, t, hT1[t][:])
                        else:
                            # L2 tail: z = dinv*(h2@W3b); pool accumulations
                            hTc = wpool.tile([H, 128], BF16, tag="hTc")
                            nc.scalar.activation(hTc[:], ps_t[:],
                                                 mybir.ActivationFunctionType.Copy)
                            ps_y = psY.tile([128, H], F32, tag="y", name="ps_y")
                            nc.tensor.matmul(ps_y[:], hTc[:], Wb[2][:],
                                             start=True, stop=True)
                            z_sb = wpool.tile([128, H], BF16, tag="zsb")
                            nc.scalar.activation(z_sb[:], ps_y[:],
                                                 mybir.ActivationFunctionType.Copy,
                                                 scale=dinv_t[:, t:t + 1])
                            poh_t = ohpool.tile([128, G], BF16, tag="poh")
                            nc.sync.dma_start(out=poh_t[:],
                                              in_=P["poh"][:, t * G:(t + 1) * G])
                            nc.tensor.matmul(psum_ph[:], h_sb[:], poh_t[:],
                                             start=(t == 0), stop=(t == TILES - 1),
                                             skip_group_check=True)
                            nc.tensor.matmul(psum_pz[:], z_sb[:],
                                             wp_all[:, t * G:(t + 1) * G],
                                             start=(t == 0), stop=False,
                                             skip_group_check=True)
                assert ri == len(regions), (ri, len(regions))

            # ---------- finish pooled^T = psum_pz + W3a^T @ P2T + b3 x cnt01 ----------
            p2t_sb = spool.tile([H, G], BF16, tag="p2t")
            nc.vector.tensor_copy(p2t_sb[:], psum_ph[:])
            nc.tensor.matmul(psum_pz[:], Wa[2][:], p2t_sb[:],
                             start=False, stop=False, skip_group_check=True)
            nc.tensor.matmul(psum_pz[:], bias[2][:], cnt01_t[:],
                             start=False, stop=True, skip_group_check=True)
            pl_sb = spool.tile([H, G], F32, tag="plsb")
            nc.vector.tensor_copy(pl_sb[:], psum_pz[:])
            nc.sync.dma_start(out=pool_in[:, :], in_=pl_sb[:])
            nc.gpsimd.collective_compute(
                "AllGather", mybir.AluOpType.bypass,
                replica_groups=[list(range(cfg.ncores))],
                ins=[pool_in[:, :].opt()], outs=[pool_ag[:, :].opt()],
            )
            # local sum over the 8 gathered partials: load as [H, 8, G]
            allp = spool.tile([H, cfg.ncores * G], F32, tag="allp")
            nc.sync.dma_start(
                out=allp[:],
                in_=pool_ag[:, :].rearrange("(i h) g -> h i g", h=H))
            half = spool.tile([H, 4 * G], F32, tag="arH")
            nc.vector.tensor_tensor(
                out=half[:], in0=allp[:, 0:4 * G], in1=allp[:, 4 * G:8 * G],
                op=mybir.AluOpType.add)
            quad = spool.tile([H, 2 * G], F32, tag="arQ")
            nc.vector.tensor_tensor(
                out=quad[:], in0=half[:, 0:2 * G], in1=half[:, 2 * G:4 * G],
                op=mybir.AluOpType.add)
            arT = spool.tile([H, G], F32, tag="arT")
            nc.vector.tensor_tensor(
                out=arT[:], in0=quad[:, 0:G], in1=quad[:, G:2 * G],
                op=mybir.AluOpType.add)
            ps_yo = psY.tile([128, H], F32, tag="y", name="ps_y")
            ps_o = ps_yo[0:G, 0:C]
            nc.tensor.matmul(ps_o, arT[:], wlin_t[:], start=True, stop=False)
            nc.tensor.matmul(ps_o, ones_t[:, :G], blin_t[:], start=False, stop=True)
            out_sb = wpool.tile([G, C], F32, tag="outsb")
            nc.vector.tensor_copy(out_sb[:], ps_o)
            nc.sync.dma_start(out=out_ext[:, :], in_=out_sb[:])

    nc.compile()
    return nc


# ---------------------------------------------------------------- driver
def make_in_maps(cfg, percore, cnt01, cmx, W1, b1, W2, b2, W3, b3, Wlin, blin):
    iota = np.tile(np.arange(128, dtype=np.float32)[None, :], (128, 1))
    ident = np.eye(128, dtype=np.float32)  # identb only
    Ws = [np.asarray(W1, np.float32), np.asarray(W2, np.float32), np.asarray(W3, np.float32)]
    bs = [np.asarray(b1, np.float32), np.asarray(b2, np.float32), np.asarray(b3, np.float32)]
    bf = ml_dtypes.bfloat16
    in_maps = []
    for c in range(cfg.ncores):
        m = {
            "xT": percore["xT"][c].astype(bf),
            "idx": percore["idx"][c],
            "drel": percore["drel"][c].astype(bf),
            "Wp": percore["Wp"][c],
            "dinv": percore["dinv"][c],
            "batch": percore["batch"][c],
            "cinv": percore["cinv"][c],
            "poh": percore["poh"][c],
            "cnt01": cnt01[None, :].astype(bf),
            "iota": iota.astype(bf),
            "iotar": np.tile(iota.astype(bf), (1, cmx)),
            "identb": ident.astype(bf),
            "Wlin": np.ascontiguousarray(Wlin, dtype=np.float32),
            "blin": np.ascontiguousarray(blin, dtype=np.float32)[None, :],
        }
        for l in range(3):
            m[f"Wa{l}"] = np.ascontiguousarray(Ws[l][0]).astype(bf)
            m[f"Wb{l}"] = np.ascontiguousarray(Ws[l][1]).astype(bf)
            m[f"bias{l}"] = np.ascontiguousarray(bs[l])[None, :].astype(bf)
        in_maps.append(m)
    return in_maps


def run(cfg, inputs, trace=False):
    plan, percore, cnt01 = host_prep(cfg, inputs["x"], inputs["edge_index"], inputs["batch"])
    nc = build_program(cfg, plan)
    cmx = max(n // 128 for (_s, _o, n) in plan["regions"])
    in_maps = make_in_maps(cfg, percore, cnt01, cmx,
                           inputs["W1"], inputs["b1"], inputs["W2"], inputs["b2"],
                           inputs["W3"], inputs["b3"], inputs["Wlin"], inputs["blin"])
    res = run_bass_kernel_spmd(nc, in_maps, core_ids=list(range(cfg.ncores)), trace=trace)
    return np.asarray(res.results[0]["out"]), res


def kernel(**inputs) -> np.ndarray:
    out, _ = run(FULL, inputs, trace=False)
    return out
